# revision 2
# baseline (speedup 1.0000x reference)
"""Trainium2 Bass kernel for batched 2D nearest-neighbor retrieval.

For each predicted point, finds the nearest real point (argmin of squared
euclidean distance, computed exactly like the jax reference lowered by
neuronx-cc: d2 = RN(RN(pn+rn) - 2*cross) with cross from the PE fp32 matmul),
then gathers that real point's expression row.

Sharding: 8 cores = (batch b in 0..3) x (query half h in 0..1).
Each core handles 4096 queries vs all 8192 reals of its batch.
"""
import numpy as np
import concourse.bass as bass
import concourse.tile as tile
from concourse import bacc, mybir
from concourse.bass_utils import run_bass_kernel_spmd

f32 = mybir.dt.float32
u32 = mybir.dt.uint32

B, N, P, G = 4, 8192, 2, 512
QC = N // 2              # queries per core
NBLK = QC // 128         # 32 query blocks of 128
NT = N // 512            # 16 real tiles of 512

_cached = {}


def _build():
    nc = bacc.Bacc("TRN2", target_bir_lowering=False, debug=False)

    pred2T_d = nc.dram_tensor("pred2T", [2, QC], f32, kind="ExternalInput").ap()
    realT_d = nc.dram_tensor("realT", [2, N], f32, kind="ExternalInput").ap()
    rnb_d = nc.dram_tensor("rnb", [128, N], f32, kind="ExternalInput").ap()
    pncols_d = nc.dram_tensor("pncols", [128, NBLK], f32, kind="ExternalInput").ap()
    idx_d = nc.dram_tensor("idx", [128, NBLK], u32, kind="ExternalOutput").ap()

    with tile.TileContext(nc) as tc:
        with (
            tc.tile_pool(name="const", bufs=1) as cpool,
            tc.tile_pool(name="d2p", bufs=2) as d2pool,
            tc.tile_pool(name="small", bufs=3) as spool,
            tc.tile_pool(name="psum", bufs=8, space="PSUM") as ppool,
        ):
            pred2T_sb = cpool.tile([2, QC], f32, tag="pred2T")
            nc.sync.dma_start(pred2T_sb[:], pred2T_d[:])
            realT_sb = cpool.tile([2, N], f32, tag="realT")
            nc.sync.dma_start(realT_sb[:], realT_d[:])
            rnb_sb = cpool.tile([128, N], f32, tag="rnb")
            nc.sync.dma_start(rnb_sb[:], rnb_d[:])
            pncols_sb = cpool.tile([128, NBLK], f32, tag="pncols")
            nc.sync.dma_start(pncols_sb[:], pncols_d[:])
            zero8_sb = cpool.tile([128, 8], f32, tag="zero8")
            nc.vector.memset(zero8_sb[:], 0.0)
            idx_sb = cpool.tile([128, NBLK], u32, tag="idx")

            for i in range(NBLK):
                d2_sb = d2pool.tile([128, N], f32, tag="d2")
                pn_i = pncols_sb[:, i:i + 1]
                for j in range(NT):
                    ps = ppool.tile([128, 512], f32, tag="ps")
                    nc.tensor.matmul(
                        ps[:], pred2T_sb[:, bass.ts(i, 128)],
                        realT_sb[:, bass.ts(j, 512)], start=True, stop=True)
                    # d2 = (rn + pn) - 2*cross, bitwise-identical to the
                    # reference's RN(RN(pn+rn) - 2c)
                    nc.vector.scalar_tensor_tensor(
                        d2_sb[:, bass.ts(j, 512)],
                        rnb_sb[:, bass.ts(j, 512)], pn_i, ps[:],
                        op0=mybir.AluOpType.add, op1=mybir.AluOpType.subtract)
                g_sb = spool.tile([128, 1], f32, tag="g")
                nc.vector.tensor_reduce(
                    g_sb[:], d2_sb[:], axis=mybir.AxisListType.X,
                    op=mybir.AluOpType.min)
                g8_sb = spool.tile([128, 8], f32, tag="g8")
                nc.vector.tensor_scalar(
                    g8_sb[:], zero8_sb[:], g_sb[:, 0:1], None,
                    op0=mybir.AluOpType.add)
                scr_sb = spool.tile([128, 8], u32, tag="scr")
                nc.vector.max_index(scr_sb[:], g8_sb[:], d2_sb[:])
                nc.vector.tensor_copy(idx_sb[:, i:i + 1], scr_sb[:, 0:1])

            nc.sync.dma_start(idx_d[:], idx_sb[:])

    nc.compile()
    return nc


def kernel(predicted_positions, real_positions, real_expressions):
    pred = np.ascontiguousarray(predicted_positions, dtype=np.float32)
    real = np.ascontiguousarray(real_positions, dtype=np.float32)
    expr = np.asarray(real_expressions)

    if "nc" not in _cached:
        _cached["nc"] = _build()
    nc = _cached["nc"]

    in_maps = []
    for c in range(8):
        b, h = c // 2, c % 2
        p = pred[b, h * QC:(h + 1) * QC]                       # [QC, 2]
        pn = (p * p).sum(-1).astype(np.float32)                # [QC]
        rn = (real[b] * real[b]).sum(-1).astype(np.float32)    # [N]
        in_maps.append({
            "pred2T": np.ascontiguousarray((2.0 * p.T).astype(np.float32)),
            "realT": np.ascontiguousarray(real[b].T),
            "rnb": np.ascontiguousarray(np.broadcast_to(rn, (128, N))),
            "pncols": np.ascontiguousarray(pn.reshape(NBLK, 128).T),
        })

    _cached["last_in_maps"] = in_maps
    results = run_bass_kernel_spmd(nc, in_maps, list(range(8))).results

    out = np.empty((B, N, G), dtype=expr.dtype)
    for c in range(8):
        b, h = c // 2, c % 2
        idx = results[c]["idx"].T.reshape(QC).astype(np.int64)  # [QC]
        out[b, h * QC:(h + 1) * QC] = expr[b, idx]
    return out


# revision 5
# speedup vs baseline: 1112.9114x; 1112.9114x over previous
"""Trainium2 Bass kernel for batched 2D nearest-neighbor retrieval.

For each predicted point, finds the nearest real point (argmin of squared
euclidean distance, computed exactly like the jax reference lowered by
neuronx-cc: d2 = RN(RN(pn+rn) - 2*cross) with cross from the PE fp32 matmul),
then gathers that real point's expression row.

Sharding: 8 cores = (batch b in 0..3) x (query half h in 0..1).
Each core handles 4096 queries vs all 8192 reals of its batch.
"""
import numpy as np
import concourse.bass as bass
import concourse.tile as tile
from concourse import bacc, mybir
from concourse.bass_utils import run_bass_kernel_spmd

f32 = mybir.dt.float32
u32 = mybir.dt.uint32

B, N, P, G = 4, 8192, 2, 512
QC = N // 2              # queries per core
NBLK = QC // 128         # 32 query blocks of 128
NT = N // 512            # 16 real tiles of 512

_cached = {}


def _build():
    nc = bacc.Bacc("TRN2", target_bir_lowering=False, debug=False)

    pred2T_d = nc.dram_tensor("pred2T", [2, QC], f32, kind="ExternalInput").ap()
    realT_d = nc.dram_tensor("realT", [2, N], f32, kind="ExternalInput").ap()
    rn_d = nc.dram_tensor("rn", [1, N], f32, kind="ExternalInput").ap()
    pncols_d = nc.dram_tensor("pncols", [128, NBLK], f32, kind="ExternalInput").ap()
    idx_d = nc.dram_tensor("idx", [128, NBLK], u32, kind="ExternalOutput").ap()

    with tile.TileContext(nc) as tc:
        with (
            tc.tile_pool(name="const", bufs=1) as cpool,
            tc.tile_pool(name="d2p", bufs=2) as d2pool,
            tc.tile_pool(name="small", bufs=3) as spool,
            tc.tile_pool(name="psum", bufs=8, space="PSUM") as ppool,
        ):
            pred2T_sb = cpool.tile([2, QC], f32, tag="pred2T")
            nc.sync.dma_start(pred2T_sb[:], pred2T_d[:])
            realT_sb = cpool.tile([2, N], f32, tag="realT")
            nc.sync.dma_start(realT_sb[:], realT_d[:])
            rnb_sb = cpool.tile([128, N], f32, tag="rnb")
            nc.sync.dma_start(rnb_sb[0:1, :], rn_d[:])
            for k in range(7):  # 1 -> 128 partitions by doubling
                w = 1 << k
                nc.sync.dma_start(rnb_sb[w:2 * w, :], rnb_sb[0:w, :])
            pncols_sb = cpool.tile([128, NBLK], f32, tag="pncols")
            nc.sync.dma_start(pncols_sb[:], pncols_d[:])
            zero8_sb = cpool.tile([128, 8], f32, tag="zero8")
            nc.vector.memset(zero8_sb[:], 0.0)
            idx_sb = cpool.tile([128, NBLK], u32, tag="idx")

            for i in range(NBLK):
                d2_sb = d2pool.tile([128, N], f32, tag="d2")
                pn_i = pncols_sb[:, i:i + 1]
                for j in range(NT):
                    ps = ppool.tile([128, 512], f32, tag="ps")
                    nc.tensor.matmul(
                        ps[:], pred2T_sb[:, bass.ts(i, 128)],
                        realT_sb[:, bass.ts(j, 512)], start=True, stop=True)
                    # d2 = (rn + pn) - 2*cross, bitwise-identical to the
                    # reference's RN(RN(pn+rn) - 2c)
                    nc.vector.scalar_tensor_tensor(
                        d2_sb[:, bass.ts(j, 512)],
                        rnb_sb[:, bass.ts(j, 512)], pn_i, ps[:],
                        op0=mybir.AluOpType.add, op1=mybir.AluOpType.subtract)
                g_sb = spool.tile([128, 1], f32, tag="g")
                nc.vector.tensor_reduce(
                    g_sb[:], d2_sb[:], axis=mybir.AxisListType.X,
                    op=mybir.AluOpType.min)
                g8_sb = spool.tile([128, 8], f32, tag="g8")
                nc.vector.tensor_scalar(
                    g8_sb[:], zero8_sb[:], g_sb[:, 0:1], None,
                    op0=mybir.AluOpType.add)
                scr_sb = spool.tile([128, 8], u32, tag="scr")
                nc.vector.max_index(scr_sb[:], g8_sb[:], d2_sb[:])
                nc.vector.tensor_copy(idx_sb[:, i:i + 1], scr_sb[:, 0:1])

            nc.sync.dma_start(idx_d[:], idx_sb[:])

    nc.compile()
    return nc


def kernel(predicted_positions, real_positions, real_expressions):
    pred = np.ascontiguousarray(predicted_positions, dtype=np.float32)
    real = np.ascontiguousarray(real_positions, dtype=np.float32)
    expr = np.asarray(real_expressions)

    if "nc" not in _cached:
        _cached["nc"] = _build()
    nc = _cached["nc"]

    in_maps = []
    for c in range(8):
        b, h = c // 2, c % 2
        p = pred[b, h * QC:(h + 1) * QC]                       # [QC, 2]
        pn = (p * p).sum(-1).astype(np.float32)                # [QC]
        rn = (real[b] * real[b]).sum(-1).astype(np.float32)    # [N]
        in_maps.append({
            "pred2T": np.ascontiguousarray((2.0 * p.T).astype(np.float32)),
            "realT": np.ascontiguousarray(real[b].T),
            "rn": rn.reshape(1, N),
            "pncols": np.ascontiguousarray(pn.reshape(NBLK, 128).T),
        })

    _cached["last_in_maps"] = in_maps
    results = run_bass_kernel_spmd(nc, in_maps, list(range(8))).results

    out = np.empty((B, N, G), dtype=expr.dtype)
    for c in range(8):
        b, h = c // 2, c % 2
        idx = results[c]["idx"].T.reshape(QC).astype(np.int64)  # [QC]
        out[b, h * QC:(h + 1) * QC] = expr[b, idx]
    return out
